# revision 74
# baseline (speedup 1.0000x reference)
"""Causal single-head attention (B=4, T=4096, E=1024, H=64) on 8 TRN2 cores.

Sharding: 2 cores per batch; no collectives. Queries assigned in 256-row
half-groups with the fold {0,3}/{1,2} (mod 4) so both cores' causal
work-lists are identical (8 slots, key-group trips exactly 1..8); all
per-core variation lives in host-prepared input data (column permutation of
x^T, dsel parity scalars).

v2 vs baseline (62.2us -> 54.6us; engine busy: ACT ~41us = critical path,
PE ~37us, DMA ~25us serial, DVE ~18us). Remaining slack: ~3.6us DMA
start-latency head, ~4.4us output-DMA/drain tail, ~2.5us supply stalls.
All foreign-K passes run off wkq (Wk lives in both fused weights; no
V-weight dependency) and foreign-V is computed directly in token-major
form like v1own -- no fused [Wk|Wv] pass, no PE transposes, no staging
copies; the V weight is a 1KB Wv-only tensor DMA'd after f1, and the
wv-gated V1 producer passes are emitted AFTER the pair-1 scores (with a
deeper early AV-flush window) so they never block the exp supply chain:
- Scores run on fp8e4 K^T/Q^T with MatmulPerfMode.DoubleRow: both operands
  carry a stride-0 broadcast plane dim, so the PE contracts each value twice
  (result = 2*K^T@Q, folded into the exp scale=E^-0.5/2). Cost model: 0.5
  cycles/row -> scores PE time halves vs bf16. K/Q cast to fp8 in the
  PSUM->SBUF copies (ACT in the idle head phase, DVE later); V stays bf16
  (fp8 V or fp8 exp-weights push rel err past the 2e-2 gate; fp8 K/Q alone
  measured ~1.2e-2).
- attn@V is flipped: out[q-part, h-free=65] accumulating in PSUM across the
  item stream; lhsT=exT 128x128 block, rhs=V1[128,65] (ones column ->
  denominator). Free 65 instead of 512 halves AV PE time, kills the
  epilogue transposes, and leaves token-major output. CRITICAL: the 4
  q-blocks of a pair share one 2KB PSUM bank and start_tensor_calc marks
  the WHOLE bank pending-zero, so exactly ONE start=True per pair; other
  sub-regions are lazily zeroed on first touch. Epilogue = PSUM->SBUF copy
  (DVE) + one DMA per slot-half (pair 3 ships both halves in one DMA on
  the tail); the softmax divide happens on HOST (column 64 = denominator).
- ACT does exp only; it is the end-to-end critical path, so the schedule is
  built around feeding it: DMA order solved against the supply chain
  (queries before foreign tiles they gate), item (0,0) split three ways so
  exp starts ~5.6us in (slot-0 quarter after own(0), slot-1 after own(1),
  foreign half after foreign_k(0)), sitems 0/3 split into own/foreign-half exps (head supply /
  tail latency), pitem exps are emitted as independent a/b halves (own
  keys gated by q-tiles vs foreign keys gated by f-tiles) so a late f-tile
  never head-of-line blocks ready work in ACT's in-order queue, and the
  sitem foreign-half parity mask is folded into the exp bias (log-parity
  0/-30000) instead of a DVE multiply.
- PE p-state: the ramp clock starts at the first matmul and never resets on
  gaps, so only a short warmup burst is needed; masks and copies are placed
  to avoid head-of-line blocking in the in-order DVE/ACT queues.
"""
import numpy as np
import ml_dtypes

B, T, E, H = 4, 4096, 1024, 64
HGS = 256         # queries per slot (half-group size)
KG = 512          # keys per key-group
NSLOT = 8
NQ = NSLOT * HGS  # 2048 owned queries per core
ET = E // 128     # 8 E-tiles
NKB = T // 128    # 32 key blocks
SCALE = float(E) ** -0.5
SCALE_EXP = SCALE / 2.0   # DoubleRow broadcast planes double the dot product
N_WARM = 2        # PE warmup matmuls (start the p-state ramp clock)

HGS_A = [0, 3, 4, 7, 8, 11, 12, 15]   # core half 0: trips 1..8 in slot order
HGS_B = [1, 2, 5, 6, 9, 10, 13, 14]   # core half 1: trips 1..8 in slot order

# x^T column layout = DMA consumption order: qj = own half-group of slot j,
# fj = foreign half of key-group j
XQ_LAYOUT = ["q0", "q1", "f0", "q2", "q3", "f1", "q4", "q5",
             "f2", "f3", "q6", "q7", "f4", "f5", "f6", "f7"]
OWN_SLOT = {j: XQ_LAYOUT.index(f"q{j}") for j in range(NSLOT)}
FRN_SLOT = {j: XQ_LAYOUT.index(f"f{j}") for j in range(NSLOT)}

_cache = {}


def _bf16(a):
    return np.ascontiguousarray(a.astype(ml_dtypes.bfloat16))


def _build_graph():
    import concourse.mybir as mybir
    import concourse.tile as tile
    from concourse import bacc
    from concourse.masks import make_identity

    dt = mybir.dt
    DR = mybir.MatmulPerfMode.DoubleRow
    nc = bacc.Bacc(None, target_bir_lowering=False)
    xT_e = nc.declare_dram_parameter("xT", [E, T], dt.bfloat16, isOutput=False)
    wkv_e = nc.declare_dram_parameter("wkv", [128, ET * 128], dt.bfloat16,
                                      isOutput=False)
    wkq_e = nc.declare_dram_parameter("wkq", [128, ET * 128], dt.bfloat16,
                                      isOutput=False)
    tri_e = nc.declare_dram_parameter("tri", [128, 2 * HGS], dt.bfloat16,
                                      isOutput=False)
    # cols 0:8 = multiplicative 0/1 parity; cols 8:16 = log-parity bias
    # (0 or -30000) folded into the sitem foreign-half exp
    dsel_e = nc.declare_dram_parameter("dsel", [128, 2 * NSLOT], dt.float32,
                                       isOutput=False)
    # per slot: 2 q-blocks x (H cols + denominator); bf16 halves every
    # epilogue transfer incl. the tail-critical one (the host divide
    # upcasts; ~0.4% extra quantization inside the 2e-2 budget)
    out_e = nc.declare_dram_parameter("out", [128, NSLOT * 2 * (H + 1)],
                                      dt.bfloat16, isOutput=True)

    xT_r = xT_e.rearrange("(et p) t -> p et t", p=128)

    with tile.TileContext(nc) as tc:
        with (
            tc.tile_pool(name="singles", bufs=1) as singles,
            tc.tile_pool(name="persist", bufs=1) as persist,
        ):
            identity = singles.tile([128, 128], dt.bfloat16)
            make_identity(nc, identity)
            wkv_sb = singles.tile([128, ET, 128], dt.bfloat16)
            wkq_sb = singles.tile([128, ET, 128], dt.bfloat16)
            tri_sb = singles.tile([128, 2, HGS], dt.bfloat16)
            dsel_sb = singles.tile([128, 2 * NSLOT], dt.float32)

            # persistent activations
            k8 = persist.tile([64, T], dt.float8e4)     # K^T, all 4096 keys
            q8 = persist.tile([64, NQ], dt.float8e4)    # Q^T, own queries
            v1 = persist.tile([128, NKB, H + 1], dt.bfloat16)
            # x^T columns arrive pre-permuted into DMA-consumption order
            # (see XQ_LAYOUT); one tile, few fat DMAs -> fewer HWDGE sems
            # -> fewer all-engine sem-recycling rendezvous barriers.
            xq = persist.tile([128, ET, 16, HGS], dt.bfloat16, name="xq")

            warm_sb = singles.tile([128, 128], dt.bfloat16)
            nc.vector.memset(warm_sb, 1.0)  # PE warmup operand
            nc.vector.memset(v1[:, :, H], 1.0)  # denominator ones column

            with (
                tc.tile_pool(name="pscore", bufs=2, space="PSUM") as pscore,
                tc.tile_pool(name="paux", bufs=2, space="PSUM") as paux,
                tc.tile_pool(name="puv", bufs=2, space="PSUM") as puv,
                tc.tile_pool(name="ex", bufs=8) as expool,
                tc.tile_pool(name="vst", bufs=4) as vstpool,
                tc.tile_pool(name="osb", bufs=3) as osbpool,
            ):
                # ---- DMA issue helpers (all inputs on SP, feed order) ----
                def dma_wkq():
                    nc.sync.dma_start(
                        out=wkq_sb,
                        in_=wkq_e.rearrange("p (et m) -> p et m", et=ET))

                def dma_wkv():
                    wr = wkv_e.rearrange("p (et m) -> p et m", et=ET)
                    nc.sync.dma_start(out=wkv_sb[:, 0:4, :], in_=wr[:, 0:4, :])
                    nc.sync.dma_start(out=wkv_sb[:, 4:8, :], in_=wr[:, 4:8, :])

                def dma_tri():
                    nc.sync.dma_start(out=tri_sb,
                                      in_=tri_e.rearrange("p (r c) -> p r c", r=2))
                    nc.sync.dma_start(out=dsel_sb, in_=dsel_e[:, :])

                def xdma(lo, n=1, split=1):
                    # layout-slots [lo, lo+n) in one transfer
                    step = ET // split
                    for h in range(split):
                        nc.sync.dma_start(
                            out=xq[:, h * step:(h + 1) * step, lo:lo + n, :],
                            in_=xT_r[:, h * step:(h + 1) * step,
                                     lo * HGS:(lo + n) * HGS])

                # ---- projection passes ----
                def own(j, k_on_act=False):
                    """[Wk|Wq] over own cols of key-group j -> K^T own half +
                    Q^T of slot j (both cast fp8). In the head phase the K
                    copy rides the idle ACT so K and Q copies run in
                    parallel (scores wait on both)."""
                    xo = xq[:, :, OWN_SLOT[j], :]
                    ps = paux.tile([128, HGS], dt.float32, tag="a")
                    for et in range(ET):
                        nc.tensor.matmul(ps, lhsT=wkq_sb[:, et, :],
                                         rhs=xo[:, et, :],
                                         start=(et == 0), stop=(et == ET - 1))
                    if k_on_act:
                        nc.scalar.copy(out=k8[:, j * KG:j * KG + HGS],
                                       in_=ps[0:64, :])
                    else:
                        nc.vector.tensor_copy(out=k8[:, j * KG:j * KG + HGS],
                                              in_=ps[0:64, :])
                    nc.vector.tensor_copy(out=q8[:, j * HGS:(j + 1) * HGS],
                                          in_=ps[64:128, :])

                def foreign_k0():
                    """K^T for foreign half of group 0 via wkq (Wk lives in
                    both fused weights) -- avoids waiting on the wkv DMA in
                    the head; V comes later from foreign_v0()."""
                    xf = xq[:, :, FRN_SLOT[0], :]
                    ps = paux.tile([64, HGS], dt.float32, tag="a", name="psk0")
                    for et in range(ET):
                        nc.tensor.matmul(ps, lhsT=wkq_sb[:, et, 0:64],
                                         rhs=xf[:, et, :],
                                         start=(et == 0), stop=(et == ET - 1))
                    nc.scalar.copy(out=k8[:, HGS:KG], in_=ps)

                def foreign_v0():
                    """V^T for foreign half of group 0 via wkv -> V1."""
                    xf = xq[:, :, FRN_SLOT[0], :]
                    ps = paux.tile([64, HGS], dt.float32, tag="a", name="psv0")
                    for et in range(ET):
                        nc.tensor.matmul(ps, lhsT=wkv_sb[:, et, 64:128],
                                         rhs=xf[:, et, :],
                                         start=(et == 0), stop=(et == ET - 1))
                    vs = vstpool.tile([64, HGS], dt.bfloat16, tag="v")
                    nc.vector.tensor_copy(out=vs, in_=ps)
                    for b in range(2):
                        kb = 2 + b
                        pst = paux.tile([128, H], dt.bfloat16, tag="a",
                                        name="pst_vt")
                        nc.tensor.transpose(
                            pst, vs[:, b * 128:(b + 1) * 128],
                            identity[0:64, 0:64])
                        nc.vector.tensor_copy(out=v1[:, kb, 0:H], in_=pst)

                def v1own(j):
                    """V1 for own tokens of key-group j, directly:
                    out[tok,H] = sum_et x_blk^T.T @ Wv_et (free=64)."""
                    xo = xq[:, :, OWN_SLOT[j], :]
                    psv = paux.tile([128, 2, H], dt.float32, tag="a", name="psv")
                    for b in range(2):
                        for et in range(ET):
                            nc.tensor.matmul(
                                psv[:, b, :],
                                lhsT=xo[:, et, b * 128:(b + 1) * 128],
                                rhs=wkv_sb[:, et, 64:128],
                                start=(et == 0), stop=(et == ET - 1))
                    nc.vector.tensor_copy(out=v1[:, 4 * j:4 * j + 2, 0:H],
                                          in_=psv)

                def foreign(j, k_on_act=False):
                    """[Wk|Wv] over foreign cols of key-group j: K^T foreign
                    half (fp8) + V^T staging -> PE transposes -> V1."""
                    xf = xq[:, :, FRN_SLOT[j], :]
                    ps = paux.tile([128, HGS], dt.float32, tag="a")
                    for et in range(ET):
                        nc.tensor.matmul(ps, lhsT=wkv_sb[:, et, :],
                                         rhs=xf[:, et, :],
                                         start=(et == 0), stop=(et == ET - 1))
                    if k_on_act:
                        nc.scalar.copy(out=k8[:, j * KG + HGS:(j + 1) * KG],
                                       in_=ps[0:64, :])
                    else:
                        nc.vector.tensor_copy(
                            out=k8[:, j * KG + HGS:(j + 1) * KG],
                            in_=ps[0:64, :])
                    vs = vstpool.tile([64, HGS], dt.bfloat16, tag="v")
                    nc.vector.tensor_copy(out=vs, in_=ps[64:128, :])
                    for b in range(2):
                        kb = 4 * j + 2 + b
                        pst = paux.tile([128, H], dt.bfloat16, tag="a",
                                        name="pst_vt")
                        nc.tensor.transpose(
                            pst, vs[:, b * 128:(b + 1) * 128],
                            identity[0:64, 0:64])
                        nc.vector.tensor_copy(out=v1[:, kb, 0:H], in_=pst)

                # ---- scores (fp8 DoubleRow, broadcast planes) ----
                def dr64(ap2d, n):
                    return ap2d.unsqueeze(1).broadcast_to([64, 2, n])

                def score_block(out_ps, kb, q0, w):
                    nc.tensor.matmul(
                        out_ps, lhsT=dr64(k8[:, kb * 128:(kb + 1) * 128], 128),
                        rhs=dr64(q8[:, q0:q0 + w], w),
                        start=True, stop=True, perf_mode=DR)

                EXP = mybir.ActivationFunctionType.Exp

                # ---- attention items with PSUM-resident flipped AV ----
                uv_tiles = {}
                started = set()   # pairs whose uv bank got its single start
                pending = []      # deferred AV+epilogue actions

                def get_uv(p):
                    if p not in uv_tiles:
                        uv_tiles[p] = puv.tile([128, 4, H + 1], dt.float32,
                                               tag="u", name=f"uv{p}")
                    return uv_tiles[p]

                def av_flush():
                    exT, p, j, diag, qbs, stop_half = pending.pop(0)
                    uvp = get_uv(p)
                    for qi, qb in enumerate(qbs):
                        for kt in range(4):
                            if diag and kt == 1 and qi == 0:
                                continue  # masked-zero exT block
                            # start_tensor_calc marks the WHOLE 2KB PSUM
                            # bank pending-zero, so exactly ONE start per
                            # pair: every other sub-region is lazily zeroed
                            # on its first touch after that mark.
                            st = p not in started
                            started.add(p)
                            is_stop = (stop_half is not None and kt == 3
                                       and ((stop_half == 0 and qb < 2)
                                            or (stop_half == 1 and qb >= 2)))
                            nc.tensor.matmul(
                                uvp[:, qb, :],
                                lhsT=exT[:, kt, qi * 128:(qi + 1) * 128],
                                rhs=v1[:, 4 * j + kt, :],
                                start=st, stop=is_stop,
                                skip_group_check=True)
                    if stop_half is not None:
                        epi_half(p, stop_half)

                def epi_half(p, hh):
                    """Ship a finished pair: one PSUM->SBUF copy + one DMA
                    when its second half (the sitem) completes."""
                    if hh == 0:
                        return
                    o_sb = osbpool.tile([128, 4, H + 1], dt.bfloat16,
                                        tag="o")
                    nc.vector.tensor_copy(out=o_sb, in_=uv_tiles[p])
                    # flat contiguous APs on both sides -> 1040B descriptor
                    # runs instead of 260B (sub-512B runs pay 2x latency and
                    # 4x the descgen time)
                    nc.sync.dma_start(
                        out=out_e[:, 2 * p * 2 * (H + 1):
                                  (2 * p + 2) * 2 * (H + 1)],
                        in_=o_sb.rearrange("p a b -> p (a b)"))

                pitem_ex = {}

                def pitem_a(p, j):
                    """Own-key half (kb 0,1) of pair item (p, j): gated only
                    by q-tiles, so it can run while f(j) is still in flight."""
                    q0 = 2 * p * HGS
                    exT = expool.tile([128, 4, 2 * HGS], dt.bfloat16,
                                      tag="ex", name=f"exp{p}_{j}")
                    pitem_ex[(p, j)] = exT
                    psh = pscore.tile([128, 2, 2 * HGS], dt.float32,
                                      tag="sc", name="ps_h")
                    for rr in range(2):
                        score_block(psh[:, rr, :], 4 * j + rr, q0, 2 * HGS)
                    nc.scalar.activation(out=exT[:, 0:2, :], in_=psh,
                                         func=EXP, scale=SCALE_EXP)
                    if j == 2 * p:
                        nc.vector.tensor_mul(
                            exT[:, 0:2, 0:HGS], exT[:, 0:2, 0:HGS], tri_sb)

                def pitem_b(p, j):
                    """Foreign-key half (kb 2,3) + AV enqueue."""
                    q0 = 2 * p * HGS
                    diag = (j == 2 * p)
                    exT = pitem_ex.pop((p, j))
                    psh = pscore.tile([128, 2, 2 * HGS], dt.float32,
                                      tag="sc", name="ps_h")
                    for rr in range(2):
                        score_block(psh[:, rr, :], 4 * j + 2 + rr, q0, 2 * HGS)
                    nc.scalar.activation(out=exT[:, 2:4, :], in_=psh,
                                         func=EXP, scale=SCALE_EXP)
                    if diag:
                        nc.vector.tensor_scalar_mul(
                            exT[:, 2:4, 0:HGS], exT[:, 2:4, 0:HGS],
                            dsel_sb[:, 2 * p:2 * p + 1])
                    pending.append((exT, p, j, diag, (0, 1, 2, 3),
                                    0 if diag else None))
                    while len(pending) > 2:
                        av_flush()

                def pitem(p, j):
                    pitem_a(p, j)
                    pitem_b(p, j)

                sitem_ex = {}

                def sitem_a(p):
                    """Own (diag-triangle) half of the solo item for slot
                    2p+1: kb 0,1 of key-group 2p+1 -- needs only own(2p+1).
                    Split out so it can feed ACT before f(2p+1) lands."""
                    b = 2 * p + 1
                    exT = sitem_ex[p] = expool.tile([128, 4, HGS],
                                                    dt.bfloat16, tag="ex",
                                                    name="exs")
                    psh = pscore.tile([128, 2, HGS], dt.float32, tag="sc",
                                      name="ps_sa")
                    for r in range(2):
                        score_block(psh[:, r, :], 4 * b + r, b * HGS, HGS)
                    nc.scalar.activation(out=exT[:, 0:2, :], in_=psh,
                                         func=EXP, scale=SCALE_EXP)
                    nc.vector.tensor_mul(exT[:, 0:2, :], exT[:, 0:2, :],
                                         tri_sb)

                def sitem_b(p):
                    """Foreign half of the solo item + AV enqueue. For the
                    final pair, flush the backlog first so only S3's own AV
                    and epilogue sit behind the last exp."""
                    if p == 3:
                        while pending:
                            av_flush()
                    b = 2 * p + 1
                    exT = sitem_ex[p]
                    psh = pscore.tile([128, 2, HGS], dt.float32, tag="sc",
                                      name="ps_sb")
                    for r in range(2):
                        score_block(psh[:, r, :], 4 * b + 2 + r, b * HGS, HGS)
                    nc.scalar.activation(out=exT[:, 2:4, :], in_=psh,
                                         func=EXP, scale=SCALE_EXP,
                                         bias=dsel_sb[:, NSLOT + b:NSLOT + b + 1])
                    pending.append((exT, p, b, True, (2, 3), 1))
                    while len(pending) > 2:
                        av_flush()

                def sitem(p):
                    sitem_a(p)
                    sitem_b(p)

                def drain():
                    while pending:
                        av_flush()

                # ---- first item (pair 0, group 0) split for ACT head ----
                ex00 = {}

                def f00a():
                    # slot-0 diag quarter: kb 0,1 x q 0:256 (needs own(0))
                    ex00["t"] = expool.tile([128, 4, 2 * HGS], dt.bfloat16,
                                            tag="ex", name="ex00")
                    psh = pscore.tile([128, 2, HGS], dt.float32, tag="sc",
                                      name="ps00a")
                    for rr in range(2):
                        score_block(psh[:, rr, :], rr, 0, HGS)
                    nc.scalar.activation(out=ex00["t"][:, 0:2, 0:HGS],
                                         in_=psh, func=EXP, scale=SCALE_EXP)

                def f00b():
                    # slot-1 columns: kb 0,1 x q 256:512 (needs own(1))
                    psh = pscore.tile([128, 2, HGS], dt.float32, tag="sc",
                                      name="ps00b")
                    for rr in range(2):
                        score_block(psh[:, rr, :], rr, HGS, HGS)
                    nc.scalar.activation(out=ex00["t"][:, 0:2, HGS:2 * HGS],
                                         in_=psh, func=EXP, scale=SCALE_EXP)

                def f00c():
                    # foreign half: kb 2,3 x q 0:512 (needs foreign_k0())
                    psh = pscore.tile([128, 2, 2 * HGS], dt.float32, tag="sc",
                                      name="ps00c")
                    for rr in range(2):
                        score_block(psh[:, rr, :], 2 + rr, 0, 2 * HGS)
                    nc.scalar.activation(out=ex00["t"][:, 2:4, :], in_=psh,
                                         func=EXP, scale=SCALE_EXP)

                def f00m():
                    # masks for item (0,0), emitted late so they never
                    # head-of-line block the DVE copy queue
                    nc.vector.tensor_mul(ex00["t"][:, 0:2, 0:HGS],
                                         ex00["t"][:, 0:2, 0:HGS], tri_sb)
                    nc.vector.tensor_scalar_mul(
                        ex00["t"][:, 2:4, 0:HGS], ex00["t"][:, 2:4, 0:HGS],
                        dsel_sb[:, 0:1])
                    pending.append((ex00["t"], 0, 0, True, (0, 1, 2, 3), 0))
                    while len(pending) > 2:
                        av_flush()

                # ---- emission schedule ----
                # Input-DMA ring on SP. Order solved against the ACT supply
                # chain: F(1,0) (the bulk-unlock item) needs q0,q1,q2,q3,f0,
                # so those go first; wkv before f1 (v1own(0) gates AV(0,0)).
                dma_wkq()
                xdma(0, split=2)      # q0
                xdma(1, split=2)      # q1
                xdma(2)               # f0
                dma_tri()
                xdma(3)               # q2
                xdma(4)               # q3
                dma_wkv()
                xdma(5)               # f1
                xdma(6)               # q4
                xdma(7)               # q5
                xdma(8)               # f2
                xdma(9)               # f3
                xdma(10)              # q6
                xdma(11)              # q7
                xdma(12, 4)           # f4..f7

                # PE warmup: starts the p-state ramp clock (never resets).
                for i in range(N_WARM):
                    pw = paux.tile([128, 128], dt.float32, tag="a", name="warm")
                    nc.tensor.matmul(pw, lhsT=warm_sb, rhs=warm_sb,
                                     start=True, stop=True)

                own(0, k_on_act=True)
                f00a()
                own(1, k_on_act=True)
                f00b()
                foreign_k0()
                f00c()
                sitem_a(0)
                own(2, k_on_act=True)
                own(3, k_on_act=True)
                f00m()
                foreign_v0()
                v1own(0)
                v1own(1)
                pitem(1, 0)
                foreign(1)
                sitem_b(0)
                pitem(1, 1)
                own(4)
                own(5)
                v1own(2)
                pitem_a(1, 2)
                sitem_a(1)
                pitem(2, 0)
                pitem(2, 1)
                pitem_a(2, 2)
                sitem_a(2)
                foreign(2)
                pitem_b(1, 2)
                pitem_b(2, 2)
                foreign(3)
                sitem_b(1)
                pitem_a(2, 3)
                pitem_b(2, 3)
                own(6)
                own(7)
                v1own(3)
                pitem(3, 0)
                pitem(3, 1)
                pitem(3, 2)
                pitem(3, 3)
                foreign(4)
                v1own(4)
                pitem_a(2, 4)
                pitem_b(2, 4)
                pitem_a(3, 4)
                pitem_b(3, 4)
                foreign(5)
                v1own(5)
                sitem_b(2)
                pitem(3, 5)
                foreign(6)
                v1own(6)
                sitem_a(3)
                pitem(3, 6)
                foreign(7)
                v1own(7)
                sitem_b(3)
                drain()
    nc.compile()
    return nc


def _host_inputs(Wk, Wq, Wv):
    # device layout [p, et, m]: weight row et*128+p, col m
    wkv = _bf16(np.concatenate([Wk, Wv], axis=1)
                .reshape(ET, 128, 128).transpose(1, 0, 2).reshape(128, ET * 128))
    wkq = _bf16(np.concatenate([Wk, Wq], axis=1)
                .reshape(ET, 128, 128).transpose(1, 0, 2).reshape(128, ET * 128))
    rk = np.arange(HGS)[:, None]
    cq = np.arange(HGS)[None, :]
    tri = (rk <= cq).astype(np.float32)           # [256, 256] own triangle
    tri = _bf16(tri.reshape(2, 128, HGS).transpose(1, 0, 2).reshape(128, 2 * HGS))
    dsel = {}
    for half, hgs in ((0, HGS_A), (1, HGS_B)):
        par = [1.0 if hg % 2 == 1 else 0.0 for hg in hgs]
        logp = [0.0 if p > 0 else -30000.0 for p in par]
        d = np.array([par + logp] * 128, dtype=np.float32)
        dsel[half] = np.ascontiguousarray(d)
    return wkv, wkq, tri, dsel


def kernel(x, Wk, Wq, Wv):
    from concourse.bass_utils import run_bass_kernel_spmd

    x = np.asarray(x, dtype=np.float32)
    Wk = np.asarray(Wk, dtype=np.float32)
    Wq = np.asarray(Wq, dtype=np.float32)
    Wv = np.asarray(Wv, dtype=np.float32)

    if "nc" not in _cache:
        _cache["nc"] = _build_graph()
    nc = _cache["nc"]

    wkv, wkq, tri, dsel = _host_inputs(Wk, Wq, Wv)

    in_maps = []
    core_meta = []
    for b in range(B):
        xTb = _bf16(x[b].T)  # [E, T]
        for half, hgs in enumerate([HGS_A, HGS_B]):
            other = [HGS_A, HGS_B][1 - half]
            hg_of = {f"q{j}": hgs[j] for j in range(NSLOT)}
            hg_of.update({f"f{j}": other[j] for j in range(NSLOT)})
            xp = np.concatenate(
                [xTb[:, hg_of[s] * HGS:(hg_of[s] + 1) * HGS]
                 for s in XQ_LAYOUT], axis=1)
            in_maps.append({
                "xT": np.ascontiguousarray(xp),
                "wkv": wkv,
                "wkq": wkq,
                "tri": tri,
                "dsel": dsel[half],
            })
            core_meta.append((b, hgs))

    res = run_bass_kernel_spmd(nc, in_maps, core_ids=list(range(8)),
                               **_cache.get("run_kwargs", {}))
    _cache["last_result"] = res

    full = np.zeros((B, T, H), dtype=np.float32)
    for core, (b, hgs) in enumerate(core_meta):
        o = res.results[core]["out"]  # [128, NSLOT*2*(H+1)]
        o = np.asarray(o, dtype=np.float32).reshape(128, NSLOT, 2, H + 1)
        # query (slot s, block qb, partition p) -> s*256 + qb*128 + p
        o = o.transpose(1, 2, 0, 3).reshape(NQ, H + 1)
        vals = o[:, 0:H] / o[:, H:H + 1]
        for s, hg in enumerate(hgs):
            full[b, hg * HGS:(hg + 1) * HGS, :] = vals[s * HGS:(s + 1) * HGS, :]
    return full
